# revision 73
# baseline (speedup 1.0000x reference)
"""Multi-head attention (Keras-style, relu-activated dense projections)
for Trainium2, SPMD across 8 NeuronCores.

Problem (full shapes):
    B, S, D, H = 4, 1024, 1024, 16 ; DH = 64
    qp = relu(q @ Wq + bq); kp = relu(k @ Wk + bk); vp = relu(v @ Wv + bv)
    per head h: scores = qh @ kh^T / 8 ; attn = softmax(scores)
    out = relu(concat_h(attn @ vh) @ Wo + bo)

Sharding: core c = (batch b = c//2, head-group g = c%2). Each core computes
the 8 heads of group g for batch b end-to-end and produces the partial
output projection  attn_out_g @ Wo[g*512:(g+1)*512, :]  (no bias / relu).
Host sums the two partials per batch, adds bo, applies relu.

v2 redesign vs the 241us baseline (which ran the PE half-clocked most of
the kernel because attention serialized scores->exp->attnV per head pair):
  - all matmul operands bf16 (halves DMA bytes, enables FWL weight loads);
    PSUM accumulation stays f32.
  - emission software-pipelines the whole kernel into the ACT exp windows:
    while exp(iter i) runs, PE does the next jt's Q/K projection, V
    projection slices, attnV of earlier iters and the first output
    projection chunk. PE never idles >3.4us -> HAM stays warm.
  - softmax denominator: DVE tree-sum over key tiles, then TWO masked
    ones-matmuls broadcast Z_A/Z_B straight into a [128,512] psum tile
    (replaces onescol reduce + staging copies + K=33 broadcast matmul).
  - projection relu+bias moved off ACT onto DVE (tensor_scalar add+max),
    so ACT runs exp back-to-back.
  - DMA: few big rearranged transfers, weights issued from GpSimd queue,
    activations from Sync, ordered so the first score matmul can start
    ~6us in.
"""

import numpy as np
import ml_dtypes
from contextlib import ExitStack

import concourse.bass as bass
import concourse.mybir as mybir
import concourse.tile as tile
from concourse import bacc

B, S, D, H = 4, 1024, 1024, 16
DG = 512          # feature slice per core (8 heads)
HL = 8            # heads per core
DH = 64
P = 128
NCORES = 8
NJT = DG // P     # 4 feature tiles == head pairs
NST = S // P      # 8 sequence tiles
NDT = D // P      # 8 contraction tiles for projections
NPC = S // 512    # 2 query chunks of 512

F32 = mybir.dt.float32
BF16 = mybir.dt.bfloat16
F8 = mybir.dt.float8e4
AF = mybir.ActivationFunctionType
ALU = mybir.AluOpType
BFNP = ml_dtypes.bfloat16
# exp is computed shifted by EXPB (= exp(x)*e^EXPB) so the biggest
# attention weights stay inside fp8e4m3 range; the shift cancels in the
# softmax normalization.
EXPB = -3.0


def build_bass(has_vbias=False):
    nc = bacc.Bacc("TRN2", target_bir_lowering=False, debug=False,
                   num_devices=NCORES)

    # x/w tensors arrive host-pre-arranged in SBUF layout: [128(partition),
    # half, dt, 512] resp. [128, dt, dout] — every DMA line is one
    # contiguous per-partition run (128 descriptors, cheap issue).
    xqT = nc.dram_tensor("xqT", [P, 2, NDT, 512], BF16,
                         kind="ExternalInput").ap()
    xkT = nc.dram_tensor("xkT", [P, 2, NDT, 512], BF16,
                         kind="ExternalInput").ap()
    xvT = nc.dram_tensor("xvT", [P, 2, NDT, 512], BF16,
                         kind="ExternalInput").ap()
    # wq/wk are jt-major so the first head-pair's weight slice arrives in a
    # single small transfer; wv stays dt-major (consumed whole per dt).
    wq = nc.dram_tensor("wq", [NJT, P, NDT, P], BF16,
                        kind="ExternalInput").ap()
    wk = nc.dram_tensor("wk", [NJT, P, NDT, P], BF16,
                        kind="ExternalInput").ap()
    wv = nc.dram_tensor("wv", [P, NDT, DG], BF16, kind="ExternalInput").ap()
    bq = nc.dram_tensor("bq", [P, NJT], F32, kind="ExternalInput").ap()
    bk = nc.dram_tensor("bk", [P, NJT], F32, kind="ExternalInput").ap()
    bv = nc.dram_tensor("bv", [1, DG], BF16, kind="ExternalInput").ap()
    wo = nc.dram_tensor("wo", [P, NJT, D], BF16, kind="ExternalInput").ap()
    bcm_in = nc.dram_tensor("bcmask", [P, 2 * P], BF16,
                            kind="ExternalInput").ap()
    out = nc.dram_tensor("out", [S, D], BF16, kind="ExternalOutput").ap()

    with tile.TileContext(nc) as tc, ExitStack() as ctx, \
            nc.allow_low_precision(reason="bf16 compute is intentional"):
        consts = ctx.enter_context(tc.tile_pool(name="consts", bufs=1))
        xpool = ctx.enter_context(tc.tile_pool(name="xpool", bufs=6))
        wpool = ctx.enter_context(tc.tile_pool(name="wpool", bufs=8))
        wopool = ctx.enter_context(tc.tile_pool(name="wopool", bufs=1))
        qkpool = ctx.enter_context(tc.tile_pool(name="qkpool", bufs=1))
        vpool = ctx.enter_context(tc.tile_pool(name="vpool", bufs=1))
        epool = ctx.enter_context(tc.tile_pool(name="epool", bufs=4))
        aopool = ctx.enter_context(tc.tile_pool(name="aopool", bufs=1))
        t1pool = ctx.enter_context(tc.tile_pool(name="t1pool", bufs=1))
        espool = ctx.enter_context(tc.tile_pool(name="espool", bufs=2))
        rpool = ctx.enter_context(tc.tile_pool(name="rpool", bufs=4))
        outpool = ctx.enter_context(tc.tile_pool(name="outpool", bufs=2))

        psA = ctx.enter_context(tc.tile_pool(name="psA", bufs=2, space="PSUM"))
        psP = ctx.enter_context(tc.tile_pool(name="psP", bufs=2, space="PSUM"))
        psB = ctx.enter_context(tc.tile_pool(name="psB", bufs=1, space="PSUM"))
        psD = ctx.enter_context(tc.tile_pool(name="psD", bufs=1, space="PSUM"))

        # --- loads spread across the three DMA rings (SP+ACT are HWDGE;
        # GpSimd is SWDGE with slow issue but an otherwise-idle ring). Each
        # ring is FIFO at ~90GB/s, so in-ring order = deadline order, and
        # the first-exp critical path (wk-jt0, xk0, wq-jt0, xq0) leads.
        bqT = consts.tile([P, NJT], F32, tag="bqT")
        nc.scalar.dma_start(out=bqT, in_=bq)
        bkT = consts.tile([P, NJT], F32, tag="bkT")
        nc.scalar.dma_start(out=bkT, in_=bk)
        bcm = consts.tile([P, 2 * P], BF16, tag="bcm")
        nc.scalar.dma_start(out=bcm, in_=bcm_in)
        expb = consts.tile([P, 1], F32, tag="expb")
        nc.vector.memset(expb, EXPB)
        # HAM warmup: ~3.8us of garbage matmuls on the const tile as soon
        # as it lands, so the PE clock-gate opens to 8/8 while the real
        # input DMAs are still streaming. Output is never read.
        warm = psD.tile([P, 256], F32, tag="po")
        for i in range(32):
            nc.tensor.matmul(warm, lhsT=bcm[:, 0:P], rhs=bcm,
                             start=(i == 0), stop=(i == 31),
                             skip_group_check=True)
        # one tile per jt slice so the first LDWEIGHTS only waits on the
        # first 256KB transfer (multiple DMAs into one tile coarsen deps).
        def load_wjt(src, jt, eng):
            t = wpool.tile([P, NDT, P], BF16, tag="w")
            eng.dma_start(out=t, in_=src[jt])
            return t

        # Critical ramp set: each ring's FIFO head caps at ~90GB/s, so the
        # first-exp transfers are split ACROSS rings (halves of each x tile
        # on SP and ACT, first weight slices on the idle GpSimd ring), and
        # nothing else is in flight until they are through.
        wkt = [load_wjt(wk, 0, nc.gpsimd), None, None, None]
        wqt = [load_wjt(wq, 0, nc.gpsimd), None, None, None]

        def load_x_split(src, half):
            t = xpool.tile([P, NDT, 512], BF16, tag="x")
            nc.sync.dma_start(out=t[:, 0:NDT // 2, :],
                              in_=src[:, half, 0:NDT // 2])
            nc.scalar.dma_start(out=t[:, NDT // 2:NDT, :],
                                in_=src[:, half, NDT // 2:NDT])
            return t

        xk = [load_x_split(xkT, 0), None]
        xq = [load_x_split(xqT, 0), None]
        xk[1] = load_x_split(xkT, 1)
        xv = [None, None]
        if has_vbias:
            # broadcast bv across partitions for the DVE bias-add
            bvb = consts.tile([P, DG], BF16, tag="bvb")
            nc.scalar.dma_start(out=bvb, in_=bv.to_broadcast([P, DG]))

        # --- persistent activations ---------------------------------------
        qpT = qkpool.tile([P, NJT, S], BF16, tag="qpT")
        kpT = qkpool.tile([P, NJT, S], BF16, tag="kpT")
        vpa = vpool.tile([P, NST, DG], BF16, tag="vpa")
        aoT3 = aopool.tile([P, NJT, S], BF16, tag="aoT3")

        # ------------------------------------------------------------------
        def emit_qkproj(wt, xs, bT, dst, jt, half):
            """dst[:, jt, half*512:] = relu(w[:,jt-cols].T @ x[half] + b)"""
            ps = psP.tile([P, 512], F32, tag="pp")
            for dt_ in range(NDT):
                nc.tensor.matmul(
                    ps, lhsT=wt[jt][:, dt_, :],
                    rhs=xs[half][:, dt_, :],
                    start=(dt_ == 0), stop=(dt_ == NDT - 1))
            nc.vector.tensor_scalar(
                dst[:, jt, half * 512:(half + 1) * 512], ps,
                scalar1=bT[:, jt:jt + 1], scalar2=0.0,
                op0=ALU.add, op1=ALU.max)

        def emit_vproj(st):
            """vpa[:, st, :] = relu(x_v[st-cols].T @ wv + bv), in fp8"""
            ps = psP.tile([P, 512], F32, tag="pp")
            half, q = st // 4, st % 4
            for dt_ in range(NDT):
                nc.tensor.matmul(
                    ps, lhsT=xv[half][:, dt_, q * P:(q + 1) * P],
                    rhs=wvt[:, dt_, :],
                    start=(dt_ == 0), stop=(dt_ == NDT - 1))
            if has_vbias:
                nc.vector.tensor_add(ps, ps, bvb)
            nc.vector.tensor_scalar_max(vpa[:, st, :], ps, 0.0)

        def emit_scores_exp(pc, hp, ex=None, uts=range(NST)):
            """returns ex [128(k), ut, 1024] bf16 (head A cols 0:512, B 512:)"""
            if ex is None:
                ex = epool.tile([P, NST, 1024], BF16, tag="exp")
            pslice = slice(pc * 512, (pc + 1) * 512)
            for ut in uts:
                uslice = slice(ut * P, (ut + 1) * P)
                pw = psA.tile([P, 1024], F32, tag="ps")
                nc.tensor.matmul(
                    pw[:, 0:512],
                    lhsT=kpT[0:DH, hp, uslice], rhs=qpT[0:DH, hp, pslice],
                    start=True, stop=True)
                nc.tensor.matmul(
                    pw[:, 512:1024],
                    lhsT=kpT[DH:P, hp, uslice], rhs=qpT[DH:P, hp, pslice],
                    start=True, stop=True)
                nc.scalar.activation(ex[:, ut, :], pw, AF.Exp,
                                     scale=0.125, bias=expb)
            return ex

        def emit_finz(ex):
            """softmax denominators: rcp [128,512] f32, rows 0:64 = 1/Z_A
            broadcast, rows 64:128 = 1/Z_B. Partition reduction runs on the
            (otherwise idle) GpSimd engine, keeping it off the PE queue."""
            t1 = t1pool.tile([P, 4, 1024], BF16, tag="t1")
            nc.vector.tensor_add(t1, ex[:, 0:4, :], ex[:, 4:8, :])
            nc.vector.tensor_add(t1[:, 0:2, :], t1[:, 0:2, :], t1[:, 2:4, :])
            exsum = espool.tile([P, 1024], BF16, tag="exsum")
            nc.vector.tensor_add(exsum, t1[:, 0, :], t1[:, 1, :])
            # masked ones-matmuls reduce partitions AND broadcast Z in one
            # step: rows 0:64 <- Z_A, rows 64:128 <- Z_B. Shares psB's bank
            # with nt (both drain quickly into DVE).
            zps = psB.tile([P, 512], F32, tag="nt")
            nc.tensor.matmul(zps, lhsT=bcm[:, 0:P], rhs=exsum[:, 0:512],
                             start=True, stop=False)
            nc.tensor.matmul(zps, lhsT=bcm[:, P:2 * P], rhs=exsum[:, 512:1024],
                             start=False, stop=True)
            rcp = rpool.tile([P, 512], F32, tag="rcp")
            nc.vector.reciprocal_approx_fast(rcp, zps)
            return rcp

        def emit_attnv(pc, hp, ex, rcp):
            hA, hB = 2 * hp, 2 * hp + 1
            nt = psB.tile([P, 512], F32, tag="nt")
            for ut in range(NST):
                nc.tensor.matmul(
                    nt[0:DH, :],
                    lhsT=vpa[:, ut, hA * DH:(hA + 1) * DH],
                    rhs=ex[:, ut, 0:512],
                    start=(ut == 0), stop=(ut == NST - 1),
                    skip_group_check=True)
                nc.tensor.matmul(
                    nt[DH:P, :],
                    lhsT=vpa[:, ut, hB * DH:(hB + 1) * DH],
                    rhs=ex[:, ut, 512:1024],
                    start=(ut == 0), stop=(ut == NST - 1),
                    skip_group_check=True)
            nc.vector.tensor_mul(aoT3[:, hp, pc * 512:(pc + 1) * 512], nt, rcp)

        def emit_outproj(pt, copy_eng, tail=False):
            os_ = outpool.tile([P, 1024], BF16, tag="os")
            for jj in range(2):
                # psP's banks are free between projection groups: alternate
                # with psD so consecutive output groups double-buffer.
                pool = psP if jj == 1 else psD
                po_ = pool.tile([P, 512], F32, tag="pp" if pool is psP
                                else "po")
                for hp in range(NJT):
                    nc.tensor.matmul(
                        po_, lhsT=aoT3[:, hp, pt * P:(pt + 1) * P],
                        rhs=wo3[:, hp, jj * 512:(jj + 1) * 512],
                        start=(hp == 0), stop=(hp == NJT - 1))
                if copy_eng == "scalar":
                    nc.scalar.copy(os_[:, jj * 512:(jj + 1) * 512], po_)
                else:
                    nc.vector.tensor_copy(os_[:, jj * 512:(jj + 1) * 512], po_)
                # half-tile DMAs overlap the next matmul group; in the tail
                # ACT's ring is idle too, so spread across both HWDGE rings.
                eng = nc.scalar if (tail and jj == 1) else nc.sync
                eng.dma_start(
                    out=out[pt * P:(pt + 1) * P, jj * 512:(jj + 1) * 512],
                    in_=os_[:, jj * 512:(jj + 1) * 512])

        # --- software-pipelined emission ----------------------------------
        # Scores for pc-chunk 0 only need the pc0 half of qpT, so pc1-half
        # q-projections are deferred to W4+; each window's PE work is sized
        # to fit under one 8-exp ACT block (~9.2us).
        # W0: minimum work before the first exp can start — scores over the
        # first 4 key tiles only need the pc0 half of kpT jt0.
        emit_qkproj(wkt, xk, bkT, kpT, 0, 0)
        # bulk loads enter the DMA pipe only once the ramp-critical set is
        # through: a dummy GpSimd read of kpT gates its ring's issues.
        gate = consts.tile([1, 8], BF16, tag="gate")
        nc.gpsimd.tensor_copy(gate, kpT[0:1, 0, 0:8])

        def load_x_gps(src, half):
            t = xpool.tile([P, NDT, 512], BF16, tag="x")
            nc.gpsimd.dma_start(out=t, in_=src[:, half])
            return t

        wvt = wopool.tile([P, NDT, DG], BF16, tag="wv")
        wkt[1] = load_wjt(wk, 1, nc.gpsimd)
        wqt[1] = load_wjt(wq, 1, nc.gpsimd)
        xq[1] = load_x_gps(xqT, 1)
        nc.gpsimd.dma_start(out=wvt, in_=wv)
        xv[0] = load_x_gps(xvT, 0)
        wkt[2] = load_wjt(wk, 2, nc.gpsimd)
        wqt[2] = load_wjt(wq, 2, nc.gpsimd)
        xv[1] = load_x_gps(xvT, 1)
        wkt[3] = load_wjt(wk, 3, nc.gpsimd)
        wqt[3] = load_wjt(wq, 3, nc.gpsimd)
        wo3 = wopool.tile([P, NJT, D], BF16, tag="wo3")
        nc.gpsimd.dma_start(out=wo3, in_=wo)
        emit_qkproj(wqt, xq, bqT, qpT, 0, 0)
        ex00 = emit_scores_exp(0, 0, uts=range(0, 4))
        emit_qkproj(wkt, xk, bkT, kpT, 0, 1)
        emit_scores_exp(0, 0, ex=ex00, uts=range(4, NST))
        # W1 (under exp(0,0))
        emit_qkproj(wkt, xk, bkT, kpT, 1, 0)
        emit_qkproj(wkt, xk, bkT, kpT, 1, 1)
        emit_qkproj(wqt, xq, bqT, qpT, 1, 0)
        emit_vproj(0)
        r00 = emit_finz(ex00)
        ex01 = emit_scores_exp(0, 1)
        # W2 (under exp(0,1))
        emit_qkproj(wkt, xk, bkT, kpT, 2, 0)
        emit_qkproj(wkt, xk, bkT, kpT, 2, 1)
        emit_qkproj(wqt, xq, bqT, qpT, 2, 0)
        emit_vproj(1)
        emit_vproj(2)
        r01 = emit_finz(ex01)
        ex02 = emit_scores_exp(0, 2)
        # W3 (under exp(0,2)): last k-proj frees two x slots -> xq half1.
        emit_qkproj(wkt, xk, bkT, kpT, 3, 0)
        emit_qkproj(wkt, xk, bkT, kpT, 3, 1)
        emit_qkproj(wqt, xq, bqT, qpT, 3, 0)
        emit_vproj(3)
        emit_vproj(4)
        r02 = emit_finz(ex02)
        ex03 = emit_scores_exp(0, 3)
        # W4 (under exp(0,3)): finish V proj, first attnV, start pc1 q-proj.
        emit_vproj(5)
        emit_vproj(6)
        emit_vproj(7)
        emit_qkproj(wqt, xq, bqT, qpT, 0, 1)
        emit_attnv(0, 0, ex00, r00)
        r03 = emit_finz(ex03)
        ex10 = emit_scores_exp(1, 0)
        # W5 (under exp(1,0))
        emit_attnv(0, 1, ex01, r01)
        emit_attnv(0, 2, ex02, r02)
        emit_qkproj(wqt, xq, bqT, qpT, 1, 1)
        r10 = emit_finz(ex10)
        ex11 = emit_scores_exp(1, 1)
        # W6 (under exp(1,1))
        emit_attnv(0, 3, ex03, r03)
        emit_qkproj(wqt, xq, bqT, qpT, 2, 1)
        emit_outproj(0, "vector")
        emit_outproj(1, "vector")
        r11 = emit_finz(ex11)
        ex12 = emit_scores_exp(1, 2)
        # W7 (under exp(1,2))
        emit_attnv(1, 0, ex10, r10)
        emit_qkproj(wqt, xq, bqT, qpT, 3, 1)
        emit_outproj(2, "vector")
        emit_outproj(3, "vector")
        r12 = emit_finz(ex12)
        ex13 = emit_scores_exp(1, 3)
        # W8 (under exp(1,3))
        emit_attnv(1, 1, ex11, r11)
        emit_attnv(1, 2, ex12, r12)
        r13 = emit_finz(ex13)
        # tail
        emit_attnv(1, 3, ex13, r13)
        for pt in range(4, 8):
            emit_outproj(pt, "scalar", tail=True)

    nc.compile()
    return nc


_CACHE = {}


def get_nc(has_vbias=False):
    if has_vbias not in _CACHE:
        _CACHE[has_vbias] = build_bass(has_vbias)
    return _CACHE[has_vbias]


def make_bcmask():
    m = np.zeros((P, 2 * P), np.float32)
    m[:, 0:DH] = 1.0          # bcmA: out rows 0:64  <- Z_A
    m[:, P + DH:2 * P] = 1.0  # bcmB: out rows 64:128 <- Z_B
    return m.astype(BFNP)


def make_in_maps(q, k, v, Wq, bq, Wk, bk, Wv, bv, Wo, bo):
    q = np.asarray(q, np.float32)
    k = np.asarray(k, np.float32)
    v = np.asarray(v, np.float32)
    Wq = np.asarray(Wq, np.float32)
    Wk = np.asarray(Wk, np.float32)
    Wv = np.asarray(Wv, np.float32)
    Wo = np.asarray(Wo, np.float32)
    bq = np.asarray(bq, np.float32)
    bk = np.asarray(bk, np.float32)
    bv = np.asarray(bv, np.float32)

    def prep_x(x):
        # x[b] is [S, D]; device wants xT in SBUF layout [128(p within dt),
        # half, dt, 512(seq)] where source row index = dt*128 + p.
        xT = x.T.astype(BFNP)                       # [D, S]
        a = xT.reshape(NDT, P, 2, 512)              # [dt, p, half, s]
        return np.ascontiguousarray(a.transpose(1, 2, 0, 3))

    def prep_w(w):
        # w slice is [D, DG] -> [128, dt, DG], row = dt*128 + p
        a = w.astype(BFNP).reshape(NDT, P, DG)
        return np.ascontiguousarray(a.transpose(1, 0, 2))

    def prep_w_jt(w):
        # w slice is [D, DG] -> [jt, 128, dt, 128]: one contiguous block
        # per head-pair so the first slice arrives in a 256KB transfer.
        a = w.astype(BFNP).reshape(NDT, P, NJT, P)
        return np.ascontiguousarray(a.transpose(2, 1, 0, 3))

    def prep_wo(w):
        # w slice is [DG, D] -> [128, hp, D], row = hp*128 + p
        a = w.astype(BFNP).reshape(NJT, P, D)
        return np.ascontiguousarray(a.transpose(1, 0, 2))

    def prep_b(b_):
        # [DG] -> [128, jt]: partition-major layout for the per-partition
        # DVE bias operand (tiny, but keeps the DMA contiguous).
        return np.ascontiguousarray(b_.reshape(NJT, P).T)

    qP = [prep_x(q[b]) for b in range(B)]
    kP = [prep_x(k[b]) for b in range(B)]
    vP = [prep_x(v[b]) for b in range(B)]
    bcm = make_bcmask()

    in_maps = []
    for c in range(NCORES):
        b, g = divmod(c, 2)
        sl = slice(g * DG, (g + 1) * DG)
        in_maps.append({
            "xqT": qP[b],
            "xkT": kP[b],
            "xvT": vP[b],
            "wq": prep_w_jt(Wq[:, sl]),
            "wk": prep_w_jt(Wk[:, sl]),
            "wv": prep_w(Wv[:, sl]),
            "bq": prep_b(bq[sl]),
            "bk": prep_b(bk[sl]),
            "bv": np.ascontiguousarray(bv[sl]).reshape(1, DG).astype(BFNP),
            "wo": prep_wo(Wo[sl, :]),
            "bcmask": bcm,
        })
    return in_maps


def combine_outputs(parts, bo):
    bo = np.asarray(bo, np.float32)
    out = np.empty((B, S, D), np.float32)
    for b in range(B):
        pa = np.asarray(parts[2 * b]).astype(np.float32)
        pb = np.asarray(parts[2 * b + 1]).astype(np.float32)
        out[b] = np.maximum(pa + pb + bo[None, :], 0.0)
    return out


def run(in_maps, trace=False, has_vbias=False, **kwargs):
    from concourse.bass_utils import run_bass_kernel_spmd
    nc = get_nc(has_vbias)
    return run_bass_kernel_spmd(nc, in_maps, list(range(NCORES)),
                                trace=trace, **kwargs)


def kernel(q, k, v, Wq, bq, Wk, bk, Wv, bv, Wo, bo):
    in_maps = make_in_maps(q, k, v, Wq, bq, Wk, bk, Wv, bv, Wo, bo)
    res = run(in_maps, has_vbias=bool(np.any(np.asarray(bv))))
    parts = [res.results[c]["out"] for c in range(NCORES)]
    return combine_outputs(parts, bo)
